# revision 27
# baseline (speedup 1.0000x reference)
"""BoxFilter (9x9 box-sum, clamped borders) Trainium2 Bass kernel.

Input  x: [16, 3, 1024, 1024] f32, r=4 (hardcoded).
Output y: same shape; y[b,c,i,j] = sum of x[b,c,u,v] over the
(2r+1)x(2r+1) window centered at (i,j), clipped to the image bounds
(exactly what the reference's cumsum+diff computes).

Sharding: pure data parallel over 8 cores, 6 of the 48 images each.

Error budget: tolerance is rel < 2e-2 of max|y| (~78) i.e. ~1.5 abs.
All HBM traffic is bf16 (x sent as bf16, y written as bf16 and upcast
on the host); measured end-to-end error ~4e-3.

The W-direction 9-window sum runs as ONE merged tensor_tensor_scan
per 128-row block on the DVE (~2.14 ns/step, dtype-independent; no
DVE perf modes exist on HW and ladder/presum variants all cost >=
the scan), using full 128-partition blocks (8 scans/image = the
minimum). Every 5th block's W-pass is OFFLOADED to the TensorEngine
as 9 shift-accumulated identity matmuls over the padded row tile
(~4.3us vs the scan's 2.19us, but the PE has slack), which balances
DVE (~86us), PE (~82us) and DMA (~82us). Offloaded W-matmuls are
emitted 5 blocks late (the PE runs in order, so the next blocks'
H-passes must come first or the DVE starves while the PE chews the
18 W streams).

Per-core pipeline (per image):
  - 9 input slabs of <=128 rows at 120-row offsets stream in as bf16.
  - Each 128-row OUTPUT block j accumulates TWO banded bf16 matmuls
    per 512-col PSUM bank (window rows below 120(j+1) come from slab
    j, the rest from slab j+1), so PSUM collects all 128 H-filtered
    rows despite the 8-row halo exceeding one slab's 128 rows.
  - One ScalarEngine copy PSUM f32 -> SBUF bf16 per block, into a
    per-block region of one persistent SBUF strip with 9 leading and
    4 trailing zero columns (all pads zeroed by two strided memsets
    in the prologue; no pool-slot reuse, so the pads are valid under
    any schedule).
  - W direction: merged scan, state = (y[t] + state) - y[t-9] over
    1028 steps (fp32 state): box_end[t] = sum_{k=max(0,t-8)}^{t} y[k];
    the leading zero pad gives the left clamp, and the last 4 steps
    (data0 = trailing zeros, data1 = y[W-9..W-6]) walk the right
    clamp down from box_end[W-1]. Output col j (j < W-r) = bx[j+r].
  - Input DMAs on the Sync queue, output DMAs on the GpSimd queue.
  - A 1-element scalar copy at t=0 preloads the ACT table off the
    critical path.
"""

import os
import numpy as np
import ml_dtypes

from concourse import bass, mybir, tile, bacc
from concourse.bass_utils import run_bass_kernel_spmd

F32 = mybir.dt.float32
BF16 = mybir.dt.bfloat16
H, W = 1024, 1024
N_CORES = 8
IPC = 6  # images per core: (16*3)/8
R = 4
D = 2 * R + 1  # 9

N_SLABS = 9   # input slabs per image, row0 = 120*i, <=128 rows
N_BLOCKS = 8  # output blocks per image, rows [128j, 128j+128)


def _slab_rows(i):
    row0 = 120 * i
    return row0, min(128, H - row0)


def _band_matrices() -> np.ndarray:
    """bands[:, (2j+w)*128 : (2j+w+1)*128] maps input slab (j+w) rows to
    output block j rows: entry [r, c] = 1 iff input row row0+r falls in
    the clamped 9-window of output row 128j+c, with window rows split
    between the two slabs at absolute row 120(j+1). The last 128 cols
    hold an identity matrix (lhsT for the PE W-pass of offloaded
    blocks)."""
    bands = np.zeros((128, N_BLOCKS * 2 * 128 + 128), ml_dtypes.bfloat16)
    bands[:, N_BLOCKS * 2 * 128 :] = np.eye(128, dtype=ml_dtypes.bfloat16)
    for j in range(N_BLOCKS):
        split = 120 * (j + 1)
        for w, slab in ((0, j), (1, j + 1)):
            row0, nrows = _slab_rows(slab)
            col0 = (2 * j + w) * 128
            for c in range(128):
                h_out = 128 * j + c
                lo = max(0, h_out - R)
                hi = min(H - 1, h_out + R)
                for u in range(lo, hi + 1):
                    if (u < split) == (w == 0):
                        r = u - row0
                        if 0 <= r < nrows:
                            bands[r, col0 + c] = 1.0
    return bands


_CACHE: dict = {}

# Set by the most recent kernel() call (for test harnesses).
LAST_RESULTS = None


def _build():
    nc = bacc.Bacc(
        "TRN2", target_bir_lowering=False, debug=False, enable_asserts=False
    )
    x_d = nc.dram_tensor("x_b", [IPC, H, W], BF16, kind="ExternalInput").ap()
    bands_d = nc.dram_tensor(
        "bands", [128, N_BLOCKS * 2 * 128 + 128], BF16, kind="ExternalInput"
    ).ap()
    y_d = nc.dram_tensor("y", [IPC, H, W], BF16, kind="ExternalOutput").ap()

    ADD = mybir.AluOpType.add
    SUB = mybir.AluOpType.subtract

    with tile.TileContext(nc) as tc:
        with (
            tc.tile_pool(name="const", bufs=1) as const_pool,
            tc.tile_pool(name="xin", bufs=10) as in_pool,
            tc.tile_pool(name="ps", bufs=3, space="PSUM") as ps_pool,
            tc.tile_pool(name="pw", bufs=1, space="PSUM") as pw_pool,
            tc.tile_pool(name="box", bufs=6) as box_pool,
        ):
            # Preload the ACT table before the first real scalar copy.
            warm = const_pool.tile([1, 1], F32)
            nc.vector.memset(warm[:], 0.0)
            warm2 = const_pool.tile([1, 1], BF16)
            nc.scalar.copy(warm2[:], warm[:])

            # Bands stream just-in-time on the sync queue: block j's two
            # 128-col bands right after its second input slab, so block 0
            # is never gated on the full band tensor.
            bands_t = const_pool.tile([128, N_BLOCKS * 2 * 128 + 128], BF16)

            # Blocks whose W-pass runs on the TensorEngine instead of the
            # DVE scan, to balance the two engines (DVE scan = 2.19us,
            # PE 9-stream pass = ~4.3us; PE has ~55us of slack).
            offload_idx = {2, 6, 11, 15, 20, 24, 29, 33, 38, 42}

            # One persistent SBUF strip holds every block's padded
            # H-filtered tile at a fixed 1040-col pitch: no pool-slot
            # reuse, so the pad zeros written here can never be exposed
            # to an unwritten slot by the scheduler. Two strided memsets
            # zero all 48 left pads ([yb, yb+9)) and right pads.
            import bass_rust as _br
            NB = IPC * N_BLOCKS
            PITCH = 1040
            yt_all = const_pool.tile([128, NB * PITCH], BF16)
            padl = yt_all[:, 0:1]
            padl.ap = _br.VecI64Pair([[NB * PITCH, 128], [PITCH, NB], [1, D]])
            nc.vector.memset(padl, 0.0)
            padr = yt_all[:, D + W : D + W + 1]
            padr.ap = _br.VecI64Pair([[NB * PITCH, 128], [PITCH, NB], [1, R]])
            nc.vector.memset(padr, 0.0)

            blk_idx = 0
            deferred = []
            for img in range(IPC):
                xts = []
                for s in range(N_SLABS):
                    row0, nrows = _slab_rows(s)
                    xt = in_pool.tile([128, W], BF16, tag="xin")
                    nc.sync.dma_start(
                        xt[:nrows], x_d[img, row0 : row0 + nrows, :]
                    )
                    xts.append((xt, nrows))
                    if img == 0 and s <= 8:
                        lo, hi = (2048, 2176) if s == 8 else (256 * s, 256 * (s + 1))
                        nc.sync.dma_start(
                            bands_t[:, lo:hi], bands_d[:, lo:hi]
                        )

                for j in range(N_BLOCKS):
                    # yt strip region: [yb+0:yb+9) zeros, [yb+9:yb+1033)
                    # = H-filtered rows, [yb+1033:yb+1037) zeros (drive the
                    # right-border scan steps)
                    yb = blk_idx * PITCH

                    # The first and last blocks of the whole stream run as
                    # two bank-aligned half scans: the first scan then only
                    # waits on bank-0 matmuls + a 512-col copy (shorter
                    # critical path into the scan phase), and the final
                    # output DMA halves (left half issued a scan earlier).
                    split = blk_idx == 0 or blk_idx == IPC * N_BLOCKS - 1

                    ps = ps_pool.tile([128, 1024], F32, tag="ps")
                    for h in range(2):
                        for w in (0, 1):
                            xt, nrows = xts[j + w]
                            band_ap = bands_t[
                                :nrows, (2 * j + w) * 128 : (2 * j + w + 1) * 128
                            ]
                            nc.tensor.matmul(
                                ps[:, h * 512 : (h + 1) * 512],
                                lhsT=band_ap,
                                rhs=xt[:nrows, h * 512 : (h + 1) * 512],
                                start=(w == 0),
                                stop=(w == 1),
                            )
                        if split:
                            nc.scalar.copy(
                                yt_all[:, yb + D + h * 512 : yb + D + (h + 1) * 512],
                                ps[:, h * 512 : (h + 1) * 512],
                            )
                    if not split:
                        nc.scalar.copy(yt_all[:, yb + D : yb + D + W], ps[:, :])

                    if blk_idx in offload_idx:
                        # PE W-pass: box9 = 9 shift-accumulated identity
                        # matmuls over the padded yt (zero pads supply the
                        # clamp), freeing the DVE scan for this block. The
                        # PE runs in order, so DEFER the W streams by two
                        # blocks: the next scans' H-passes are emitted
                        # first and the DVE keeps banked scans to run
                        # while the PE chews the 18 W streams.
                        def emit_w(img=img, j=j, yb=yb):
                            ident = bands_t[:, 2048:2176]
                            pw = pw_pool.tile([128, 1024], F32, tag="pw")
                            for h in range(2):
                                for s in range(D):
                                    c0 = 5 + s + h * 512
                                    nc.tensor.matmul(
                                        pw[:, h * 512 : (h + 1) * 512],
                                        lhsT=ident,
                                        rhs=yt_all[:, yb + c0 : yb + c0 + 512],
                                        start=(s == 0),
                                        stop=(s == D - 1),
                                    )
                            bo = box_pool.tile([128, W], BF16, tag="boxo")
                            nc.scalar.copy(bo[:, :], pw[:, :])
                            nc.gpsimd.dma_start(
                                y_d[img, 128 * j : 128 * (j + 1), :], bo[:, :]
                            )

                        deferred.append((blk_idx + 6, emit_w))
                        blk_idx += 1
                        while deferred and deferred[0][0] <= blk_idx:
                            deferred.pop(0)[1]()
                        continue

                    # Merged scan: state = (y[t] + state) - y[t-9], 1028
                    # steps; output col c (c < W-r) = bx[c+r], the last r
                    # cols come from the trailing clamp walk.
                    bx = box_pool.tile([128, W + R], BF16, tag="box")
                    if split:
                        # Left: steps 0..511 -> out cols 0..507.
                        nc.vector.tensor_tensor_scan(
                            bx[:, 0:512],
                            yt_all[:, yb + D : yb + D + 512],
                            yt_all[:, yb : yb + 512],
                            0.0,
                            op0=ADD,
                            op1=SUB,
                        )
                        nc.gpsimd.dma_start(
                            y_d[img, 128 * j : 128 * (j + 1), 0:508],
                            bx[:, R : R + 508],
                        )
                        # Right: steps 512..1027, state chained through
                        # initial = box_end[511] -> out cols 508..1023.
                        nc.vector.tensor_tensor_scan(
                            bx[:, 512 : 512 + 516],
                            yt_all[:, yb + D + 512 : yb + D + W + R],
                            yt_all[:, yb + 512 : yb + W + R],
                            bx[:, 511:512],
                            op0=ADD,
                            op1=SUB,
                        )
                        nc.gpsimd.dma_start(
                            y_d[img, 128 * j : 128 * (j + 1), 508:1024],
                            bx[:, 512 : 512 + 516],
                        )
                    else:
                        nc.vector.tensor_tensor_scan(
                            bx[:, 0 : W + R],
                            yt_all[:, yb + D : yb + D + W + R],
                            yt_all[:, yb : yb + W + R],
                            0.0,
                            op0=ADD,
                            op1=SUB,
                        )
                        nc.gpsimd.dma_start(
                            y_d[img, 128 * j : 128 * (j + 1), :],
                            bx[:, R : R + W],
                        )
                    blk_idx += 1
                    while deferred and deferred[0][0] <= blk_idx:
                        deferred.pop(0)[1]()

            while deferred:
                deferred.pop(0)[1]()

    nc.compile()
    return nc


def kernel(x: np.ndarray, r) -> np.ndarray:
    global LAST_RESULTS
    x = np.asarray(x, dtype=np.float32)
    assert x.shape == (16, 3, H, W), x.shape
    assert int(r) == R, r

    nc = _CACHE.get("nc")
    if nc is None:
        nc = _CACHE["nc"] = _build()

    xb = np.ascontiguousarray(x.reshape(N_CORES, IPC, H, W)).astype(
        ml_dtypes.bfloat16
    )
    bands = _band_matrices()
    in_maps = [{"x_b": xb[c], "bands": bands} for c in range(N_CORES)]

    trace = bool(int(os.environ.get("BOX_TRACE", "0")))
    tmpdir = os.environ.get("BOX_TRACE_DIR") or None
    if tmpdir:
        os.makedirs(tmpdir, exist_ok=True)
    res = run_bass_kernel_spmd(
        nc, in_maps, list(range(N_CORES)), trace=trace, tmpdir=tmpdir
    )
    LAST_RESULTS = res
    y = np.stack([res.results[c]["y"] for c in range(N_CORES)])
    return y.reshape(16, 3, H, W).astype(np.float32)


# revision 28
# speedup vs baseline: 1.1913x; 1.1913x over previous
"""BoxFilter (9x9 box-sum, clamped borders) Trainium2 Bass kernel.

Input  x: [16, 3, 1024, 1024] f32, r=4 (hardcoded).
Output y: same shape; y[b,c,i,j] = sum of x[b,c,u,v] over the
(2r+1)x(2r+1) window centered at (i,j), clipped to the image bounds
(exactly what the reference's cumsum+diff computes).

Sharding: pure data parallel over 8 cores, 6 of the 48 images each.

Error budget: tolerance is rel < 2e-2 of max|y| (~78) i.e. ~1.5 abs.
All HBM traffic is bf16 (x sent as bf16, y written as bf16 and upcast
on the host); measured end-to-end error ~4e-3.

The W-direction 9-window sum runs as ONE merged tensor_tensor_scan
per 128-row block on the DVE (~2.14 ns/step, dtype-independent; no
DVE perf modes exist on HW and ladder/presum variants all cost >=
the scan), using full 128-partition blocks (8 scans/image = the
minimum). Every 5th block's W-pass is OFFLOADED to the TensorEngine
as 9 shift-accumulated identity matmuls over the padded row tile
(~4.3us vs the scan's 2.19us, but the PE has slack), which balances
DVE (~86us), PE (~82us) and DMA (~82us). Offloaded W-matmuls are
emitted 5 blocks late (the PE runs in order, so the next blocks'
H-passes must come first or the DVE starves while the PE chews the
18 W streams).

Per-core pipeline (per image):
  - 9 input slabs of <=128 rows at 120-row offsets stream in as bf16.
  - Each 128-row OUTPUT block j accumulates TWO banded bf16 matmuls
    per 512-col PSUM bank (window rows below 120(j+1) come from slab
    j, the rest from slab j+1), so PSUM collects all 128 H-filtered
    rows despite the 8-row halo exceeding one slab's 128 rows.
  - One ScalarEngine copy PSUM f32 -> SBUF bf16 per block, into a
    per-block region of one persistent SBUF strip with 9 leading and
    4 trailing zero columns (all pads zeroed by two strided memsets
    in the prologue; no pool-slot reuse, so the pads are valid under
    any schedule).
  - W direction: merged scan, state = (y[t] + state) - y[t-9] over
    1028 steps (fp32 state): box_end[t] = sum_{k=max(0,t-8)}^{t} y[k];
    the leading zero pad gives the left clamp, and the last 4 steps
    (data0 = trailing zeros, data1 = y[W-9..W-6]) walk the right
    clamp down from box_end[W-1]. Output col j (j < W-r) = bx[j+r].
  - Input DMAs on the Sync queue, output DMAs on the GpSimd queue.
  - A 1-element scalar copy at t=0 preloads the ACT table off the
    critical path.
"""

import os
import numpy as np
import ml_dtypes

from concourse import bass, mybir, tile, bacc
from concourse.bass_utils import run_bass_kernel_spmd

F32 = mybir.dt.float32
BF16 = mybir.dt.bfloat16
H, W = 1024, 1024
N_CORES = 8
IPC = 6  # images per core: (16*3)/8
R = 4
D = 2 * R + 1  # 9

N_SLABS = 9   # input slabs per image, row0 = 120*i, <=128 rows
N_BLOCKS = 8  # output blocks per image, rows [128j, 128j+128)


def _slab_rows(i):
    row0 = 120 * i
    return row0, min(128, H - row0)


def _band_matrices() -> np.ndarray:
    """bands[:, (2j+w)*128 : (2j+w+1)*128] maps input slab (j+w) rows to
    output block j rows: entry [r, c] = 1 iff input row row0+r falls in
    the clamped 9-window of output row 128j+c, with window rows split
    between the two slabs at absolute row 120(j+1). The last 128 cols
    hold an identity matrix (lhsT for the PE W-pass of offloaded
    blocks)."""
    bands = np.zeros((128, N_BLOCKS * 2 * 128 + 128), ml_dtypes.bfloat16)
    bands[:, N_BLOCKS * 2 * 128 :] = np.eye(128, dtype=ml_dtypes.bfloat16)
    for j in range(N_BLOCKS):
        split = 120 * (j + 1)
        for w, slab in ((0, j), (1, j + 1)):
            row0, nrows = _slab_rows(slab)
            col0 = (2 * j + w) * 128
            for c in range(128):
                h_out = 128 * j + c
                lo = max(0, h_out - R)
                hi = min(H - 1, h_out + R)
                for u in range(lo, hi + 1):
                    if (u < split) == (w == 0):
                        r = u - row0
                        if 0 <= r < nrows:
                            bands[r, col0 + c] = 1.0
    return bands


_CACHE: dict = {}

# Set by the most recent kernel() call (for test harnesses).
LAST_RESULTS = None


def _build():
    nc = bacc.Bacc(
        "TRN2", target_bir_lowering=False, debug=False, enable_asserts=False
    )
    x_d = nc.dram_tensor("x_b", [IPC, H, W], BF16, kind="ExternalInput").ap()
    bands_d = nc.dram_tensor(
        "bands", [128, N_BLOCKS * 2 * 128 + 128], BF16, kind="ExternalInput"
    ).ap()
    y_d = nc.dram_tensor("y", [IPC, H, W], BF16, kind="ExternalOutput").ap()

    ADD = mybir.AluOpType.add
    SUB = mybir.AluOpType.subtract

    with tile.TileContext(nc) as tc:
        with (
            tc.tile_pool(name="const", bufs=1) as const_pool,
            tc.tile_pool(name="xin", bufs=10) as in_pool,
            tc.tile_pool(name="ps", bufs=3, space="PSUM") as ps_pool,
            tc.tile_pool(name="pw", bufs=1, space="PSUM") as pw_pool,
            tc.tile_pool(name="box", bufs=6) as box_pool,
        ):
            # Preload the ACT table before the first real scalar copy.
            warm = const_pool.tile([1, 1], F32)
            nc.vector.memset(warm[:], 0.0)
            warm2 = const_pool.tile([1, 1], BF16)
            nc.scalar.copy(warm2[:], warm[:])

            # Bands stream just-in-time on the sync queue: block j's two
            # 128-col bands right after its second input slab, so block 0
            # is never gated on the full band tensor.
            bands_t = const_pool.tile([128, N_BLOCKS * 2 * 128 + 128], BF16)

            # Blocks whose W-pass runs on the TensorEngine instead of the
            # DVE scan, to balance the two engines (DVE scan = 2.19us,
            # PE 9-stream pass = ~4.3us; PE has ~55us of slack).
            offload_idx = {2, 7, 12, 17, 22, 27, 32, 37, 42}

            # One persistent SBUF strip holds every block's padded
            # H-filtered tile at a fixed 1040-col pitch: no pool-slot
            # reuse, so the pad zeros written here can never be exposed
            # to an unwritten slot by the scheduler. Two strided memsets
            # zero all 48 left pads ([yb, yb+9)) and right pads.
            import bass_rust as _br
            NB = IPC * N_BLOCKS
            PITCH = 1040
            yt_all = const_pool.tile([128, NB * PITCH], BF16)
            padl = yt_all[:, 0:1]
            padl.ap = _br.VecI64Pair([[NB * PITCH, 128], [PITCH, NB], [1, D]])
            nc.vector.memset(padl, 0.0)
            padr = yt_all[:, D + W : D + W + 1]
            padr.ap = _br.VecI64Pair([[NB * PITCH, 128], [PITCH, NB], [1, R]])
            nc.vector.memset(padr, 0.0)

            blk_idx = 0
            deferred = []
            for img in range(IPC):
                xts = []
                for s in range(N_SLABS):
                    row0, nrows = _slab_rows(s)
                    xt = in_pool.tile([128, W], BF16, tag="xin")
                    nc.sync.dma_start(
                        xt[:nrows], x_d[img, row0 : row0 + nrows, :]
                    )
                    xts.append((xt, nrows))
                    if img == 0 and s <= 8:
                        lo, hi = (2048, 2176) if s == 8 else (256 * s, 256 * (s + 1))
                        nc.sync.dma_start(
                            bands_t[:, lo:hi], bands_d[:, lo:hi]
                        )

                for j in range(N_BLOCKS):
                    # yt strip region: [yb+0:yb+9) zeros, [yb+9:yb+1033)
                    # = H-filtered rows, [yb+1033:yb+1037) zeros (drive the
                    # right-border scan steps)
                    yb = blk_idx * PITCH

                    # The first and last blocks of the whole stream run as
                    # two bank-aligned half scans: the first scan then only
                    # waits on bank-0 matmuls + a 512-col copy (shorter
                    # critical path into the scan phase), and the final
                    # output DMA halves (left half issued a scan earlier).
                    split = blk_idx == 0 or blk_idx == IPC * N_BLOCKS - 1

                    ps = ps_pool.tile([128, 1024], F32, tag="ps")
                    for h in range(2):
                        for w in (0, 1):
                            xt, nrows = xts[j + w]
                            band_ap = bands_t[
                                :nrows, (2 * j + w) * 128 : (2 * j + w + 1) * 128
                            ]
                            nc.tensor.matmul(
                                ps[:, h * 512 : (h + 1) * 512],
                                lhsT=band_ap,
                                rhs=xt[:nrows, h * 512 : (h + 1) * 512],
                                start=(w == 0),
                                stop=(w == 1),
                            )
                        if split:
                            nc.scalar.copy(
                                yt_all[:, yb + D + h * 512 : yb + D + (h + 1) * 512],
                                ps[:, h * 512 : (h + 1) * 512],
                            )
                    if not split:
                        nc.scalar.copy(yt_all[:, yb + D : yb + D + W], ps[:, :])

                    if blk_idx in offload_idx:
                        # PE W-pass: box9 = 9 shift-accumulated identity
                        # matmuls over the padded yt (zero pads supply the
                        # clamp), freeing the DVE scan for this block. The
                        # PE runs in order, so DEFER the W streams by two
                        # blocks: the next scans' H-passes are emitted
                        # first and the DVE keeps banked scans to run
                        # while the PE chews the 18 W streams.
                        def emit_w(img=img, j=j, yb=yb):
                            ident = bands_t[:, 2048:2176]
                            pw = pw_pool.tile([128, 1024], F32, tag="pw")
                            for h in range(2):
                                for s in range(D):
                                    c0 = 5 + s + h * 512
                                    nc.tensor.matmul(
                                        pw[:, h * 512 : (h + 1) * 512],
                                        lhsT=ident,
                                        rhs=yt_all[:, yb + c0 : yb + c0 + 512],
                                        start=(s == 0),
                                        stop=(s == D - 1),
                                    )
                            bo = box_pool.tile([128, W], BF16, tag="boxo")
                            nc.scalar.copy(bo[:, :], pw[:, :])
                            nc.gpsimd.dma_start(
                                y_d[img, 128 * j : 128 * (j + 1), :], bo[:, :]
                            )

                        deferred.append((blk_idx + 6, emit_w))
                        blk_idx += 1
                        while deferred and deferred[0][0] <= blk_idx:
                            deferred.pop(0)[1]()
                        continue

                    # Merged scan: state = (y[t] + state) - y[t-9], 1028
                    # steps; output col c (c < W-r) = bx[c+r], the last r
                    # cols come from the trailing clamp walk.
                    bx = box_pool.tile([128, W + R], BF16, tag="box")
                    if split:
                        # Left: steps 0..511 -> out cols 0..507.
                        nc.vector.tensor_tensor_scan(
                            bx[:, 0:512],
                            yt_all[:, yb + D : yb + D + 512],
                            yt_all[:, yb : yb + 512],
                            0.0,
                            op0=ADD,
                            op1=SUB,
                        )
                        nc.gpsimd.dma_start(
                            y_d[img, 128 * j : 128 * (j + 1), 0:508],
                            bx[:, R : R + 508],
                        )
                        # Right: steps 512..1027, state chained through
                        # initial = box_end[511] -> out cols 508..1023.
                        nc.vector.tensor_tensor_scan(
                            bx[:, 512 : 512 + 516],
                            yt_all[:, yb + D + 512 : yb + D + W + R],
                            yt_all[:, yb + 512 : yb + W + R],
                            bx[:, 511:512],
                            op0=ADD,
                            op1=SUB,
                        )
                        nc.gpsimd.dma_start(
                            y_d[img, 128 * j : 128 * (j + 1), 508:1024],
                            bx[:, 512 : 512 + 516],
                        )
                    else:
                        nc.vector.tensor_tensor_scan(
                            bx[:, 0 : W + R],
                            yt_all[:, yb + D : yb + D + W + R],
                            yt_all[:, yb : yb + W + R],
                            0.0,
                            op0=ADD,
                            op1=SUB,
                        )
                        nc.gpsimd.dma_start(
                            y_d[img, 128 * j : 128 * (j + 1), :],
                            bx[:, R : R + W],
                        )
                    blk_idx += 1
                    while deferred and deferred[0][0] <= blk_idx:
                        deferred.pop(0)[1]()

            while deferred:
                deferred.pop(0)[1]()

    nc.compile()
    return nc


def kernel(x: np.ndarray, r) -> np.ndarray:
    global LAST_RESULTS
    x = np.asarray(x, dtype=np.float32)
    assert x.shape == (16, 3, H, W), x.shape
    assert int(r) == R, r

    nc = _CACHE.get("nc")
    if nc is None:
        nc = _CACHE["nc"] = _build()

    xb = np.ascontiguousarray(x.reshape(N_CORES, IPC, H, W)).astype(
        ml_dtypes.bfloat16
    )
    bands = _band_matrices()
    in_maps = [{"x_b": xb[c], "bands": bands} for c in range(N_CORES)]

    trace = bool(int(os.environ.get("BOX_TRACE", "0")))
    tmpdir = os.environ.get("BOX_TRACE_DIR") or None
    if tmpdir:
        os.makedirs(tmpdir, exist_ok=True)
    res = run_bass_kernel_spmd(
        nc, in_maps, list(range(N_CORES)), trace=trace, tmpdir=tmpdir
    )
    LAST_RESULTS = res
    y = np.stack([res.results[c]["y"] for c in range(N_CORES)])
    return y.reshape(16, 3, H, W).astype(np.float32)
